# revision 1
# baseline (speedup 1.0000x reference)
"""ContentOnlyRouter MoE kernel for 8x TRN2 NeuronCores.

Strategy (expert-parallel, two SPMD launches):
  Launch A (data-parallel over tokens): each core scores its 2048-token shard
    against sign(tile_sigs) and computes per-token argmax expert ids.
    Scoring uses a bf16 hi/lo split of x (products with +-1 are exact in bf16;
    fp32 PSUM accumulation) so the argmax matches fp32 scoring exactly.
  Host glue: stable counting-sort of the 16384 expert ids (64KB of metadata)
    to build per-expert gather lists.
  Launch B (expert-parallel): core t owns expert t. dma_gather(transpose=True)
    pulls its ~2048 assigned token rows from a replicated bf16 copy of x and
    transposes them on the fly into [d, tok] matmul layout. 8 accumulating
    bf16 matmuls per 128-token block compute x @ W[t], bias added on DVE,
    fp32 rows stored compactly. Host scatters rows back to token order.

Shapes are hardcoded for B=4, S=4096, D=1024, T=8 per the problem spec.
"""

import os

os.environ.setdefault("JAX_PLATFORMS", "")

import numpy as np
import ml_dtypes

import concourse.bass as bass
import concourse.bacc as bacc
import concourse.mybir as mybir
import concourse.tile as tile
from concourse.masks import make_identity

B, S, D, T = 4, 4096, 1024, 8
NTOK = B * S            # 16384 tokens
NG = 4                  # score groups of 512 tokens per shard
NCORES = 8
SHARD = NTOK // NCORES  # 2048 tokens scored per core
CAP = 2304              # per-expert token capacity (18 blocks of 128)
GCHUNK = 384            # tokens per dma_gather call (3 blocks of 128)
NCHUNK = CAP // GCHUNK  # 6
TRASH = NTOK            # row index used for padding slots
DC = D // 128           # 8 contraction chunks

F32 = mybir.dt.float32
BF16 = mybir.dt.bfloat16
I16 = mybir.dt.int16

_perf = []  # exec_time_ns per launch when tracing


def build_launch_a(iters=1):
    """Scores + argmax for one 2048-token shard."""
    nc = bacc.Bacc(None)
    xht = nc.dram_tensor("xht", [128, DC, SHARD], BF16, kind="ExternalInput")
    xlt = nc.dram_tensor("xlt", [128, DC, SHARD], BF16, kind="ExternalInput")
    sgn = nc.dram_tensor("sgn", [128, DC, T], BF16, kind="ExternalInput")
    idx = nc.dram_tensor("idx", [SHARD], F32, kind="ExternalOutput")

    with tile.TileContext(nc) as tc:
        with (
            tc.tile_pool(name="const", bufs=1) as const,
            tc.tile_pool(name="xa", bufs=4) as xa,
            tc.tile_pool(name="ps", bufs=2, space="PSUM") as ps,
            tc.tile_pool(name="pst", bufs=4, space="PSUM") as pst,
            tc.tile_pool(name="sb", bufs=2) as sb,
        ):
            sgn_sb = const.tile([128, DC, T], BF16)
            nc.sync.dma_start(out=sgn_sb, in_=sgn[:, :, :])
            ident = const.tile([128, 128], F32)
            make_identity(nc, ident)
            # rev-iota: value 7-t at expert slot t (first-occurrence argmax)
            revio = const.tile([128, NG * 4, T], F32)
            for t in range(T):
                nc.vector.memset(revio[:, :, t : t + 1], float(T - 1 - t))
            sc_all = const.tile([128, NG * 4, T], F32)

            import contextlib
            loop = tc.For_i(0, iters, 1) if iters > 1 else contextlib.nullcontext()
            with loop:
                self_body_a(nc, tc, xa, ps, pst, sb, sgn_sb, ident, revio, sc_all, xht, xlt, idx)
    nc.compile()
    return nc


def self_body_a(nc, tc, xa, ps, pst, sb, sgn_sb, ident, revio, sc_all, xht, xlt, idx):
    if True:
            for g in range(NG):
                xh_g = xa.tile([128, DC, 512], BF16, tag="xh")
                xl_g = xa.tile([128, DC, 512], BF16, tag="xl")
                nc.sync.dma_start(out=xh_g, in_=xht[:, :, 512 * g : 512 * (g + 1)])
                nc.sync.dma_start(out=xl_g, in_=xlt[:, :, 512 * g : 512 * (g + 1)])
                psum_s = ps.tile([T, 512], F32)
                for c in range(DC):
                    nc.tensor.matmul(
                        out=psum_s,
                        lhsT=sgn_sb[:, c, :],
                        rhs=xh_g[:, c, :],
                        start=(c == 0),
                        stop=False,
                    )
                for c in range(DC):
                    nc.tensor.matmul(
                        out=psum_s,
                        lhsT=sgn_sb[:, c, :],
                        rhs=xl_g[:, c, :],
                        start=False,
                        stop=(c == DC - 1),
                    )
                s_sb = sb.tile([T, 512], F32)
                nc.vector.tensor_copy(out=s_sb, in_=psum_s)
                for j in range(4):
                    p_t = pst.tile([128, T], F32)
                    nc.tensor.transpose(
                        out=p_t,
                        in_=s_sb[:, 128 * j : 128 * (j + 1)],
                        identity=ident[0:T, 0:T],
                    )
                    nc.vector.tensor_copy(out=sc_all[:, 4 * g + j, :], in_=p_t)

            # argmax over the last axis (8 experts) per token
            smax = sb.tile([128, NG * 4, 1], F32, tag="smax")
            nc.vector.reduce_max(out=smax, in_=sc_all, axis=mybir.AxisListType.X)
            m = sb.tile([128, NG * 4, T], F32, tag="m")
            nc.vector.tensor_tensor(
                out=m,
                in0=sc_all,
                in1=smax.to_broadcast([128, NG * 4, T]),
                op=mybir.AluOpType.is_ge,
            )
            nc.vector.tensor_tensor(out=m, in0=m, in1=revio, op=mybir.AluOpType.mult)
            mm = sb.tile([128, NG * 4, 1], F32, tag="mm")
            nc.vector.reduce_max(out=mm, in_=m, axis=mybir.AxisListType.X)
            idxv = sb.tile([128, NG * 4], F32, tag="idxv")
            nc.vector.tensor_scalar(
                out=idxv,
                in0=mm[:, :, 0],
                scalar1=-1.0,
                scalar2=float(T - 1),
                op0=mybir.AluOpType.mult,
                op1=mybir.AluOpType.add,
            )
            # token n = 128*q + p  ->  idx[n]
            nc.sync.dma_start(
                out=idx.rearrange("(q p) -> p q", p=128), in_=idxv
            )


def build_launch_b(iters=1):
    """Gather + expert matmul for one expert's tokens."""
    nc = bacc.Bacc(None)
    xfull = nc.dram_tensor("xfull", [NTOK + 1, D], BF16, kind="ExternalInput")
    wt = nc.dram_tensor("wt", [128, DC, D], BF16, kind="ExternalInput")
    bt = nc.dram_tensor("bt", [D], F32, kind="ExternalInput")
    gl = nc.dram_tensor("gl", [128, CAP // 16], I16, kind="ExternalInput")
    orows = nc.dram_tensor("orows", [CAP, D], F32, kind="ExternalOutput")

    with tile.TileContext(nc) as tc:
        with (
            tc.tile_pool(name="const", bufs=1) as const,
            tc.tile_pool(name="gx", bufs=3) as gxp,
            tc.tile_pool(name="ps", bufs=4, space="PSUM") as ps,
            tc.tile_pool(name="osb", bufs=3) as osb,
        ):
            w_sb = const.tile([128, DC, D], BF16)
            nc.sync.dma_start(out=w_sb, in_=wt[:, :, :])
            b_sb = const.tile([128, D], F32)
            bt_ap = bt[:]
            nc.gpsimd.dma_start(
                out=b_sb,
                in_=bass.AP(
                    tensor=bt_ap.tensor, offset=bt_ap.offset,
                    ap=[[0, 128]] + list(bt_ap.ap),
                ),
            )
            gl_sb = const.tile([128, CAP // 16], I16)
            nc.sync.dma_start(out=gl_sb, in_=gl[:, :])

            import contextlib
            loop = tc.For_i(0, iters, 1) if iters > 1 else contextlib.nullcontext()
            with loop:
                self_body_b(nc, tc, gxp, ps, osb, w_sb, b_sb, gl_sb, xfull, orows)
    nc.compile()
    return nc


def self_body_b(nc, tc, gxp, ps, osb, w_sb, b_sb, gl_sb, xfull, orows):
    if True:
            for ch in range(NCHUNK):
                gx = gxp.tile([128, DC, GCHUNK], BF16)
                nc.gpsimd.dma_gather(
                    out_ap=gx,
                    in_ap=xfull[:, :],
                    idxs_ap=gl_sb[:, (GCHUNK // 16) * ch : (GCHUNK // 16) * (ch + 1)],
                    num_idxs=GCHUNK,
                    num_idxs_reg=GCHUNK,
                    elem_size=D,
                    transpose=True,
                )
                for blk in range(GCHUNK // 128):
                    tok = slice(128 * blk, 128 * (blk + 1))
                    ps0 = ps.tile([128, 512], F32, tag="ps0")
                    ps1 = ps.tile([128, 512], F32, tag="ps1")
                    for c in range(DC):
                        nc.tensor.matmul(
                            out=ps0,
                            lhsT=gx[:, c, tok],
                            rhs=w_sb[:, c, 0:512],
                            start=(c == 0),
                            stop=(c == DC - 1),
                        )
                        nc.tensor.matmul(
                            out=ps1,
                            lhsT=gx[:, c, tok],
                            rhs=w_sb[:, c, 512:1024],
                            start=(c == 0),
                            stop=(c == DC - 1),
                        )
                    o_t = osb.tile([128, D], F32)
                    nc.vector.tensor_add(out=o_t[:, 0:512], in0=ps0, in1=b_sb[:, 0:512])
                    nc.vector.tensor_add(out=o_t[:, 512:1024], in0=ps1, in1=b_sb[:, 512:1024])
                    row0 = GCHUNK * ch + 128 * blk
                    nc.sync.dma_start(out=orows[row0 : row0 + 128, :], in_=o_t)


_nc_a = None
_nc_b = None


def _get_programs():
    global _nc_a, _nc_b
    if _nc_a is None:
        _nc_a = build_launch_a()
        _nc_b = build_launch_b()
    return _nc_a, _nc_b


def _run_spmd(nc, in_maps, label):
    if os.environ.get("BASS_SIM"):
        from concourse.bass_interp import CoreSim

        results = []
        for im in in_maps:
            sim = CoreSim(nc)
            for k, v in im.items():
                sim.tensor(k)[:] = v
            sim.simulate()
            out = {}
            for alloc in nc.m.functions[0].allocations:
                if getattr(alloc, "kind", None) == "ExternalOutput":
                    name = alloc.memorylocations[0].name
                    out[name] = np.array(sim.mem_tensor(name))
            results.append(out)

        class R:
            pass

        r = R()
        r.results = results
        r.exec_time_ns = None
        return r
    from concourse.bass_utils import run_bass_kernel_spmd

    trace = bool(os.environ.get("BASS_TRACE"))
    kw = {}
    if trace:
        tdir = os.path.abspath(f"trace_{label}")
        os.makedirs(tdir, exist_ok=True)
        kw = dict(trace=True, tmpdir=tdir, trace_cores=[0])
    res = run_bass_kernel_spmd(nc, in_maps, core_ids=list(range(NCORES)), **kw)
    if trace:
        _perf.append((label, res.exec_time_ns, res.mean_exec_time_ns))
    return res


def kernel(x, tile_sigs, W, b):
    x = np.asarray(x, np.float32)
    tile_sigs = np.asarray(tile_sigs, np.float32)
    W = np.asarray(W, np.float32)
    b = np.asarray(b, np.float32)
    _perf.clear()

    nc_a, nc_b = _get_programs()

    xf = x.reshape(NTOK, D)
    x_hi = xf.astype(ml_dtypes.bfloat16)
    x_lo = (xf - x_hi.astype(np.float32)).astype(ml_dtypes.bfloat16)
    sgn = np.sign(tile_sigs).astype(ml_dtypes.bfloat16)  # [T, D]
    # sgn_in[p, c, t] = sgn[t, 128c + p]
    sgn_in = np.ascontiguousarray(sgn.T.reshape(DC, 128, T).transpose(1, 0, 2))

    in_maps_a = []
    for c in range(NCORES):
        sh = slice(c * SHARD, (c + 1) * SHARD)
        # xht[p, ch, n] = x_hi[n, 128*ch + p]
        xht = np.ascontiguousarray(x_hi[sh].T.reshape(DC, 128, SHARD).transpose(1, 0, 2))
        xlt = np.ascontiguousarray(x_lo[sh].T.reshape(DC, 128, SHARD).transpose(1, 0, 2))
        in_maps_a.append({"xht": xht, "xlt": xlt, "sgn": sgn_in})

    res_a = _run_spmd(nc_a, in_maps_a, "a")
    idx_all = np.concatenate(
        [np.rint(res_a.results[c]["idx"]).astype(np.int64).ravel() for c in range(NCORES)]
    )

    # host routing: stable counting sort -> per-expert gather lists
    order = np.argsort(idx_all, kind="stable")
    counts = np.bincount(idx_all, minlength=T)
    assert counts.max() <= CAP, f"expert overflow: {counts}"
    bounds = np.concatenate([[0], np.cumsum(counts)])

    x_hi_full = np.vstack([x_hi, np.zeros((1, D), ml_dtypes.bfloat16)])
    gids = []
    in_maps_b = []
    for t in range(NCORES):
        ids = order[bounds[t] : bounds[t + 1]]
        glf = np.full(CAP, TRASH, np.int64)
        glf[: len(ids)] = ids
        gids.append(glf)
        wrapped = np.ascontiguousarray(
            glf.reshape(CAP // 16, 16).T.astype(np.int16)
        )  # [16, CAP//16]
        gl_in = np.tile(wrapped, (8, 1))  # replicate for 8 gpsimd cores
        # wt[p, c, e] = W[t][128c + p, e]
        wt = np.ascontiguousarray(
            W[t].astype(ml_dtypes.bfloat16).reshape(DC, 128, D).transpose(1, 0, 2)
        )
        in_maps_b.append({"xfull": x_hi_full, "wt": wt, "bt": b[t], "gl": gl_in})

    res_b = _run_spmd(nc_b, in_maps_b, "b")

    out_full = np.zeros((NTOK + 1, D), np.float32)
    for t in range(NCORES):
        out_full[gids[t]] = res_b.results[t]["orows"]
    return out_full[:NTOK].reshape(B, S, D)



# revision 2
# speedup vs baseline: 1.3503x; 1.3503x over previous
"""ContentOnlyRouter MoE kernel for 8x TRN2 NeuronCores.

Strategy (expert-parallel, two SPMD launches):
  Launch A (data-parallel over tokens): each core loads its 2048-token shard
    as fp16 in [d, token] layout and computes routing scores against
    sign(tile_sigs) with out=[128 tokens, 8 experts] matmuls (ap=8, so PE
    time is negligible; the fp16 input DMA ~11.7us is the floor).
    Raw scores are shipped to the host.
  Host glue: argmax of the fp16-accurate scores; tokens whose top-2 gap is
    < TAU are rescored exactly in float64 (a handful of tokens), making the
    routing exactly match fp32 argmax. Stable counting-sort builds per-core
    gather lists: core t owns expert t's first 2048 tokens (16 blocks) plus
    one 128-token "spare" block that holds another expert's overflow (so the
    per-core block count is 17 instead of ceil(max_count/128)=18).
  Launch B (expert-parallel): per 128-token block, dma_gather(transpose=True)
    pulls token rows from a replicated bf16 copy of x into [d, tok] layout;
    8 accumulating bf16 matmuls per 512-feature half compute x @ W. The spare
    block uses a second weight input wt2. Rows are stored compactly as bf16;
    the host converts to fp32, adds the expert bias, and scatters to token
    order.

Shapes hardcoded for B=4, S=4096, D=1024, T=8 per the problem spec.
"""

import os

os.environ.setdefault("JAX_PLATFORMS", "")

import numpy as np
import ml_dtypes

import concourse.bass as bass
import concourse.bacc as bacc
import concourse.mybir as mybir
import concourse.tile as tile

B, S, D, T = 4, 4096, 1024, 8
NTOK = B * S            # 16384 tokens
NCORES = 8
SHARD = NTOK // NCORES  # 2048 tokens scored per core
NBLK_A = SHARD // 128   # 16 score blocks per core
DC = D // 128           # 8 contraction chunks

NPRIM = 16              # primary 128-token blocks per core (expert = core id)
NSPARE = 1              # spare blocks (second expert's overflow)
TRASH = NTOK            # row index used for padding slots (zero row)
TAU = 0.10              # top-2 gap below which the host rescores exactly

F32 = mybir.dt.float32
F16 = mybir.dt.float16
BF16 = mybir.dt.bfloat16
I16 = mybir.dt.int16

_perf = []  # exec_time_ns per launch when tracing


def build_launch_a(iters=1):
    """Routing scores for one 2048-token shard; raw scores out."""
    nc = bacc.Bacc(None)
    xt = nc.dram_tensor("xt", [128, DC, SHARD], F16, kind="ExternalInput")
    sgn = nc.dram_tensor("sgn", [128, DC, T], F16, kind="ExternalInput")
    sco = nc.dram_tensor("sco", [128, NBLK_A, T], F32, kind="ExternalOutput")

    with tile.TileContext(nc) as tc:
        with (
            tc.tile_pool(name="const", bufs=1) as const,
            tc.tile_pool(name="xa", bufs=3) as xa,
            tc.tile_pool(name="ps", bufs=4, space="PSUM") as ps,
            tc.tile_pool(name="sb", bufs=2) as sb,
        ):
            sgn_sb = const.tile([128, DC, T], F16)
            nc.sync.dma_start(out=sgn_sb, in_=sgn[:, :, :])

            import contextlib
            loop = tc.For_i(0, iters, 1) if iters > 1 else contextlib.nullcontext()
            with loop:
                sc_all = sb.tile([128, NBLK_A, T], F32, tag="sc")
                for g in range(4):  # 4 chunks of 512 tokens
                    x_g = xa.tile([128, DC, 512], F16, tag="xg")
                    nc.sync.dma_start(out=x_g, in_=xt[:, :, 512 * g : 512 * (g + 1)])
                    for j in range(4):  # 4 blocks of 128 tokens
                        p = ps.tile([128, T], F32, tag="p")
                        for c in range(DC):
                            nc.tensor.matmul(
                                out=p,
                                lhsT=x_g[:, c, 128 * j : 128 * (j + 1)],
                                rhs=sgn_sb[:, c, :],
                                start=(c == 0),
                                stop=(c == DC - 1),
                            )
                        nc.vector.tensor_copy(out=sc_all[:, 4 * g + j, :], in_=p)
                nc.scalar.dma_start(out=sco[:, :, :], in_=sc_all)
    nc.compile()
    return nc


def build_launch_b(iters=1, nprim=NPRIM, nspare=NSPARE):
    """Gather + expert matmul: nprim blocks with wt, nspare blocks with wt2."""
    nblk = nprim + nspare
    cap = nblk * 128
    nc = bacc.Bacc(None)
    xfull = nc.dram_tensor("xfull", [NTOK + 1, D], BF16, kind="ExternalInput")
    wt = nc.dram_tensor("wt", [128, DC, D], BF16, kind="ExternalInput")
    wt2 = nc.dram_tensor("wt2", [128, DC, D], BF16, kind="ExternalInput")
    gl = nc.dram_tensor("gl", [128, cap // 16], I16, kind="ExternalInput")
    orows = nc.dram_tensor("orows", [cap, D], BF16, kind="ExternalOutput")

    with tile.TileContext(nc) as tc:
        with (
            tc.tile_pool(name="const", bufs=1) as const,
            tc.tile_pool(name="gx", bufs=4) as gxp,
            tc.tile_pool(name="ps", bufs=4, space="PSUM") as ps,
            tc.tile_pool(name="osb", bufs=4) as osb,
        ):
            gl_sb = const.tile([128, cap // 16], I16)
            nc.sync.dma_start(out=gl_sb, in_=gl[:, :])
            w_sb = const.tile([128, DC, D], BF16)
            # chunked load so the first matmul starts after ~1 chunk
            for c in range(DC):
                nc.sync.dma_start(out=w_sb[:, c, :], in_=wt[:, c, :])
            w2_sb = const.tile([128, DC, D], BF16)
            nc.sync.dma_start(out=w2_sb, in_=wt2[:, :, :])

            import contextlib
            loop = tc.For_i(0, iters, 1) if iters > 1 else contextlib.nullcontext()
            with loop:
                for b in range(nblk):
                    gx = gxp.tile([128, DC, 128], BF16, tag="gx")
                    nc.gpsimd.dma_gather(
                        out_ap=gx,
                        in_ap=xfull[:, :],
                        idxs_ap=gl_sb[:, 8 * b : 8 * (b + 1)],
                        num_idxs=128,
                        num_idxs_reg=128,
                        elem_size=D,
                        transpose=True,
                    )
                    w = w_sb if b < nprim else w2_sb
                    ps0 = ps.tile([128, 512], F32, tag="ps0")
                    ps1 = ps.tile([128, 512], F32, tag="ps1")
                    for c in range(DC):
                        nc.tensor.matmul(
                            out=ps0,
                            lhsT=gx[:, c, :],
                            rhs=w[:, c, 0:512],
                            start=(c == 0),
                            stop=(c == DC - 1),
                        )
                        nc.tensor.matmul(
                            out=ps1,
                            lhsT=gx[:, c, :],
                            rhs=w[:, c, 512:1024],
                            start=(c == 0),
                            stop=(c == DC - 1),
                        )
                    o_t = osb.tile([128, D], BF16, tag="ot")
                    nc.vector.tensor_copy(out=o_t[:, 0:512], in_=ps0)
                    nc.vector.tensor_copy(out=o_t[:, 512:1024], in_=ps1)
                    nc.scalar.dma_start(
                        out=orows[128 * b : 128 * (b + 1), :], in_=o_t
                    )
    nc.compile()
    return nc


_nc_a = None
_nc_b = None


def _get_programs():
    global _nc_a, _nc_b
    if _nc_a is None:
        _nc_a = build_launch_a()
        _nc_b = build_launch_b()
    return _nc_a, _nc_b


def _run_spmd(nc, in_maps, label):
    if os.environ.get("BASS_SIM"):
        from concourse.bass_interp import CoreSim

        results = []
        for im in in_maps:
            sim = CoreSim(nc)
            for k, v in im.items():
                sim.tensor(k)[:] = v
            sim.simulate()
            out = {}
            for alloc in nc.m.functions[0].allocations:
                if getattr(alloc, "kind", None) == "ExternalOutput":
                    name = alloc.memorylocations[0].name
                    out[name] = np.array(sim.mem_tensor(name))
            results.append(out)

        class R:
            pass

        r = R()
        r.results = results
        r.exec_time_ns = None
        return r
    from concourse.bass_utils import run_bass_kernel_spmd

    trace = bool(os.environ.get("BASS_TRACE"))
    kw = {}
    if trace:
        tdir = os.path.abspath(f"trace_{label}")
        os.makedirs(tdir, exist_ok=True)
        kw = dict(trace=True, tmpdir=tdir, trace_cores=[0])
    res = run_bass_kernel_spmd(nc, in_maps, core_ids=list(range(NCORES)), **kw)
    if trace:
        _perf.append((label, res.exec_time_ns, res.mean_exec_time_ns))
    return res


def _wrap_gl(glf, cap):
    wrapped = np.ascontiguousarray(glf.reshape(cap // 16, 16).T.astype(np.int16))
    return np.tile(wrapped, (8, 1))  # replicate for 8 gpsimd cores


def kernel(x, tile_sigs, W, b):
    x = np.asarray(x, np.float32)
    tile_sigs = np.asarray(tile_sigs, np.float32)
    W = np.asarray(W, np.float32)
    b = np.asarray(b, np.float32)
    _perf.clear()

    nc_a, _ = _get_programs()

    xf = x.reshape(NTOK, D)
    x16 = xf.astype(np.float16)
    sgnf = np.sign(tile_sigs).astype(np.float32)  # [T, D]
    sgn16 = sgnf.astype(np.float16)
    # sgn_in[p, c, t] = sgn[t, 128c + p]
    sgn_in = np.ascontiguousarray(sgn16.T.reshape(DC, 128, T).transpose(1, 0, 2))

    in_maps_a = []
    for c in range(NCORES):
        sh = slice(c * SHARD, (c + 1) * SHARD)
        # xt[p, ch, n] = x16[n, 128*ch + p]
        xt = np.ascontiguousarray(x16[sh].T.reshape(DC, 128, SHARD).transpose(1, 0, 2))
        in_maps_a.append({"xt": xt, "sgn": sgn_in})

    res_a = _run_spmd(nc_a, in_maps_a, "a")
    scores = np.concatenate(
        [
            res_a.results[c]["sco"].transpose(1, 0, 2).reshape(SHARD, T)
            for c in range(NCORES)
        ]
    )  # [NTOK, T], fp16-accurate
    idx_all = scores.argmax(-1)
    srt = np.sort(scores, axis=-1)
    amb = (srt[:, -1] - srt[:, -2]) < TAU
    if amb.any():
        exact = xf[amb].astype(np.float64) @ sgnf.astype(np.float64).T
        idx_all[amb] = exact.argmax(-1)

    # host routing: stable counting sort -> per-core gather lists
    order = np.argsort(idx_all, kind="stable")
    counts = np.bincount(idx_all, minlength=T)
    bounds = np.concatenate([[0], np.cumsum(counts)])

    prim_cap = NPRIM * 128
    overflow = []  # (expert, ids chunk <=128)
    prim_ids = []
    for t in range(T):
        ids = order[bounds[t] : bounds[t + 1]]
        prim_ids.append(ids[:prim_cap])
        over = ids[prim_cap:]
        for k in range(0, len(over), 128):
            overflow.append((t, over[k : k + 128]))

    if len(overflow) <= NCORES * NSPARE:
        nprim, nspare = NPRIM, NSPARE
        nc_b = _get_programs()[1]
    else:
        # pathological balance: fall back to plain expert-parallel capacity
        nprim, nspare = int(-(-counts.max() // 128)), 0
        nc_b = build_launch_b(nprim=nprim, nspare=nspare)
        prim_ids = [order[bounds[t] : bounds[t + 1]] for t in range(T)]
        overflow = []
    nblk = nprim + nspare
    cap = nblk * 128

    x_full = np.vstack([xf.astype(ml_dtypes.bfloat16),
                        np.zeros((1, D), ml_dtypes.bfloat16)])
    gids = []
    spare_expert = []
    in_maps_b = []
    for t in range(NCORES):
        glf = np.full(cap, TRASH, np.int64)
        glf[: len(prim_ids[t])] = prim_ids[t]
        if nspare and t < len(overflow):
            se, sids = overflow[t]
            glf[nprim * 128 : nprim * 128 + len(sids)] = sids
        else:
            se = t
        spare_expert.append(se)
        gids.append(glf)
        # wt[p, c, e] = W[t][128c + p, e]
        wt = np.ascontiguousarray(
            W[t].astype(ml_dtypes.bfloat16).reshape(DC, 128, D).transpose(1, 0, 2)
        )
        wt2 = np.ascontiguousarray(
            W[se].astype(ml_dtypes.bfloat16).reshape(DC, 128, D).transpose(1, 0, 2)
        )
        in_maps_b.append(
            {"xfull": x_full, "wt": wt, "wt2": wt2, "gl": _wrap_gl(glf, cap)}
        )

    res_b = _run_spmd(nc_b, in_maps_b, "b")

    out_full = np.zeros((NTOK + 1, D), np.float32)
    for t in range(NCORES):
        rows = np.asarray(res_b.results[t]["orows"]).astype(np.float32)
        pids = gids[t][: nprim * 128]
        pv = pids != TRASH
        out_full[pids[pv]] = rows[: nprim * 128][pv] + b[t]
        if nspare:
            sids = gids[t][nprim * 128 :]
            sv = sids != TRASH
            out_full[sids[sv]] = rows[nprim * 128 :][sv] + b[spare_expert[t]]
    return out_full[:NTOK].reshape(B, S, D)
